# revision 25
# baseline (speedup 1.0000x reference)
"""Single-head causal attention (B=8, T=2048, D=1024, H=64) on 8 TRN2 NeuronCores.

Sharding: data-parallel over batch B — core b computes attention for x[b].

Host-side layout choices (these dominate end-to-end time through the axon
tunnel and also cut device HBM traffic in half):
  * x is pre-cast to bf16 on the host — the device kernel computes in bf16
    anyway (matmuls bf16 with f32 PSUM accumulation), so shipping f32 and
    casting on-device wastes 2x the bytes both on the wire and in HBM.
  * Wq|Wk|Wv are pre-packed host-side into one [128, 8, 192] bf16 tensor in
    the exact SBUF layout the projection matmuls consume (partition p, D-chunk
    dc, columns 0:128 = [Wq|Wk] stacked, 128:192 = Wv) — a single contiguous
    DMA with no on-device rearrange or cast.
  * out is written bf16 (rounding adds ~0.3% L2 error, well inside tolerance)
    and widened to f32 on the host.

Per-core algorithm (all matmuls bf16 with f32 PSUM accumulation):
  1. x [T, D] bf16 is loaded per 512-row chunk, then DMA-xbar block-transposed
     into xT [D, T] in SBUF (D on partitions, 8 chunks of 128).
  2. Projections computed transposed: qT/kT/vT [H=64, T] = W.T @ x.T with the
     weight chunk as the stationary operand (PSUM accumulate over 8 D-chunks).
  3. vT is DMA-transposed back to v tiles [128, H] and augmented with a ones
     column -> v_aug [128, H+1]; the PV matmul then yields row-sums for free.
  4. Scores are computed TRANSPOSED (sT[k, q] = k @ qT, K=64 contraction) so
     the exp'd tile is directly the stationary operand of the PV matmul --
     no per-tile transpose of the probabilities is ever needed.
     Softmax skips the max-subtraction: scores*0.125 are ~N(0,1) (|s|<~7), so
     exp is numerically safe in f32/bf16. The 0.125 scale is folded into the
     ACT exp instruction. Causality: only kj<=qi blocks are computed; the
     diagonal block is masked by a 0/1 upper-triangular multiply AFTER exp.
  5. out[q, :] = (sum_k p[k,q]*v_aug[k, :]) accumulated over kj blocks in PSUM;
     final division by the row-sum (column H) happens at PSUM evacuation.
"""

import numpy as np

B, T, D, H = 8, 2048, 1024, 64
P = 128          # partition tile
NT = T // P      # 16 T-tiles
ND = D // P      # 8 D-chunks
NCORES = 8
SCALE = float(H) ** -0.5  # 0.125
SCORE_CHUNK = 1024       # PSUM score tile free size (2 banks)

_CACHE = {}


def _build_nc(reps=1):
    import concourse.bass as bass
    import concourse.tile as tile
    from concourse import bacc, mybir

    # Bacc (not Bass): its compile() runs the TRN2 sync-wait splitting pass
    # (walrus rejects multi-wait Drain instructions otherwise).
    nc = bacc.Bacc(
        "TRN2", target_bir_lowering=False, debug=False, num_devices=NCORES
    )
    f32 = mybir.dt.float32
    bf16 = mybir.dt.bfloat16

    f16 = mybir.dt.float16

    x_d = nc.declare_dram_parameter("x", [T, D], bf16, isOutput=False)
    wqkv_d = nc.declare_dram_parameter("wqkv", [P, ND, 3 * H], bf16, isOutput=False)
    mask_d = nc.declare_dram_parameter("mask", [P, P], bf16, isOutput=False)
    out_d = nc.declare_dram_parameter("out", [T, H], bf16, isOutput=True)

    ts = bass.ts
    Exp = mybir.ActivationFunctionType.Exp
    Copy = mybir.ActivationFunctionType.Copy

    with tile.TileContext(nc) as tc:
        with (
            tc.tile_pool(name="consts", bufs=1) as consts,
            tc.tile_pool(name="bigs", bufs=1) as bigs,
            tc.tile_pool(name="xstage", bufs=3) as xstage,
            tc.tile_pool(name="evac", bufs=3) as evac,
        ):
            # ---- constants (one contiguous DMA each, already in SBUF layout) ----
            wqkv_sb = consts.tile([P, ND, 3 * H], bf16)
            mask_sb = consts.tile([P, P], bf16)
            nc.sync.dma_start(wqkv_sb[:], wqkv_d[:])
            nc.sync.dma_start(mask_sb[:], mask_d[:])

            # ---- big persistent SBUF tensors ----
            xT = bigs.tile([P, ND, T], bf16)       # x transposed, [d_in_chunk, dc, t]
            # qT/kT duplicated into both partition halves (rows 0:64 == 64:128)
            # so score matmuls for two k-tiles can run CONCURRENTLY in
            # disjoint 64-row PE groups via tile_position row tiling.
            qT_sb = bigs.tile([2 * H, T], bf16)
            kT_sb = bigs.tile([2 * H, T], bf16)
            vT_sb = bigs.tile([H, T], bf16)
            # v tiles live in one [P, NT, 80] tensor: 80-element row stride
            # keeps every (t)-slice 32-byte aligned for the xbar transpose
            v_sb = bigs.tile([P, NT, 80], bf16)
            probsT = bigs.tile([P, NT, T], bf16)    # exp'd transposed scores
            ob_all = bigs.tile([P, NT, H], bf16)    # final out tiles, one store

            # ---- single-pass pipeline over 512-wide q-chunks ----
            # per chunk c: load+transpose x, project, then immediately compute
            # every score row's slice for this q-range, exp it, and run PV for
            # the q-tiles of this chunk. Attention hides in the DMA shadow of
            # later chunks' loads.
            CW = 512
            GT = 4  # T-tiles per chunk
            # PSUM budget (8 banks): proj 2 (psqk+psv, single-buffered — its
            # WAR window spans a whole chunk of score/PV work, no stall),
            # scores 3+2 (two concurrent row-tiles; deep-buffered so the PE
            # can run ahead of ACT's exp drain within a chunk — the backlog
            # clears during PV/proj phases when ACT is otherwise idle),
            # PV out 1 (its evac window spans a whole chunk, no stall).
            psum_proj = tc.alloc_tile_pool(name="psum_proj", bufs=1, space="PSUM")
            psum_sTa = tc.alloc_tile_pool(name="psum_sTa", bufs=3, space="PSUM")
            psum_sTb = tc.alloc_tile_pool(name="psum_sTb", bufs=2, space="PSUM")
            psum_out = tc.alloc_tile_pool(name="psum_out", bufs=1, space="PSUM")

            def emit_pv_chunk(c):
                # PV for all 4 q-tiles of chunk c at once, TRANSPOSED:
                # psoT[h, q] += sum_k v_aug[k, h] * probsT[k, q], accumulated
                # over all causal k-tiles in PSUM. v_aug is the (65-col)
                # stationary so every matmul streams a 128-512 wide q range —
                # ~3.4x fewer, wider matmuls than per-q-tile accumulation.
                # Row H is the ones-column product = the softmax denominator.
                nkj = GT * c + GT
                psoT = psum_out.tile([H + 1, CW], f32, tag="psoT")
                for kj in range(nkj):
                    q0 = max(P * kj, CW * c)  # causal: only q >= k columns
                    lc = CW * (c + 1) - q0
                    nc.tensor.matmul(
                        psoT[:, q0 - CW * c : q0 - CW * c + lc],
                        v_sb[:, kj, 0 : H + 1],
                        probsT[:, kj, q0 : q0 + lc],
                        start=(kj == 0),
                        stop=(kj == nkj - 1),
                    )
                # evacuate: scale into fp16 (sums stay < 2^16), xbar-transpose
                # back to [q, h] tiles, then divide by the row-sum column.
                # The 1/64 scale cancels in the division.
                pvs = evac.tile([80, CW], f16, tag="pvs")
                nc.vector.memset(pvs[:], 0.0)  # partition base must be 32-aligned
                nc.scalar.activation(
                    pvs[0 : H + 1, :], psoT[:], Copy, scale=0.015625
                )
                ptr = evac.tile([P, GT, 80], f16, tag="ptr")
                nc.sync.dma_start(ptr[:], pvs[:], transpose=True)
                for i in range(GT):
                    qi = GT * c + i
                    rs = evac.tile([P, 1], f32, tag="rs")
                    nc.vector.reciprocal(rs[:], ptr[:, i, H : H + 1])
                    nc.vector.tensor_scalar_mul(
                        ob_all[:, qi, :], ptr[:, i, 0:H], rs[:]
                    )

            for _rep in range(reps):
                for c in range(T // CW):
                    # transpose-load chunk c of x straight from DRAM:
                    # [128, 1024] bf16 rows -> xT[:, :, tile] block-transpose
                    for i in range(GT):
                        nc.sync.dma_start(
                            xT[:, :, ts(GT * c + i, P)],
                            x_d[ts(GT * c + i, P), :],
                            transpose=True,
                        )
                    psqk = psum_proj.tile([P, CW], f32, tag="psqk")
                    psv = psum_proj.tile([H, CW], f32, tag="psv")
                    for dc in range(ND):
                        st = dc == 0
                        sp = dc == ND - 1
                        nc.tensor.matmul(
                            psqk[:], wqkv_sb[:, dc, 0 : 2 * H], xT[:, dc, ts(c, CW)],
                            start=st, stop=sp,
                        )
                        nc.tensor.matmul(
                            psv[:], wqkv_sb[:, dc, 2 * H : 3 * H], xT[:, dc, ts(c, CW)],
                            start=st, stop=sp,
                        )
                    nc.vector.tensor_copy(qT_sb[0:H, ts(c, CW)], psqk[0:H, :])
                    nc.vector.tensor_copy(qT_sb[H : 2 * H, ts(c, CW)], psqk[0:H, :])
                    nc.vector.tensor_copy(kT_sb[0:H, ts(c, CW)], psqk[H : 2 * H, :])
                    nc.vector.tensor_copy(kT_sb[H : 2 * H, ts(c, CW)], psqk[H : 2 * H, :])
                    # DVE not ACT: keeps the scalar engine free for exp
                    nc.vector.tensor_copy(vT_sb[:, ts(c, CW)], psv[:])
                    # v tiles for this chunk (batched xbar transpose + ones col)
                    nc.sync.dma_start(
                        v_sb[:, GT * c : GT * (c + 1), 0:H],
                        vT_sb[:, ts(c, CW)],
                        transpose=True,
                    )
                    nc.vector.memset(v_sb[:, GT * c : GT * (c + 1), H : H + 1], 1.0)

                    # PV for the PREVIOUS chunk, emitted ahead of this chunk's
                    # scores: its matmuls keep the PE busy during the DVE
                    # evacuation of this chunk's qT/kT — otherwise the first
                    # score matmul stalls on those copies.
                    if c > 0:
                        emit_pv_chunk(c - 1)

                    # scores for every k-row intersecting this q-chunk, in
                    # pairs: the two K=64 matmuls occupy disjoint 64-row PE
                    # groups (tile_position row tiling) and run concurrently.
                    for j0 in range(0, GT * c + GT, 2):
                        emitted = []
                        for idx in (0, 1):
                            j = j0 + idx
                            q0 = max(P * j, CW * c)
                            lc = CW * (c + 1) - q0
                            bp = H * idx
                            pool = psum_sTa if idx == 0 else psum_sTb
                            sT = pool.tile([P, CW], f32, tag=f"sT{idx}")
                            nc.tensor.matmul(
                                sT[:, 0:lc],
                                kT_sb[bp : bp + H, ts(j, P)],
                                qT_sb[bp : bp + H, q0 : q0 + lc],
                                start=True,
                                stop=True,
                                tile_position=(bp, 0),
                            )
                            emitted.append((j, q0, lc, sT))
                        for j, q0, lc, sT in emitted:
                            nc.scalar.activation(
                                probsT[:, j, q0 : q0 + lc], sT[:, 0:lc], Exp,
                                scale=SCALE,
                            )
                            if j // GT == c:
                                # causal mask on diagonal block (0/1 mul after exp)
                                nc.vector.tensor_mul(
                                    probsT[:, j, P * j : P * j + P],
                                    probsT[:, j, P * j : P * j + P],
                                    mask_sb[:],
                                )
                emit_pv_chunk(T // CW - 1)

                # single batched output store
                nc.sync.dma_start(
                    out_d[:].rearrange("(t p) h -> p t h", p=P), ob_all[:]
                )
            psum_out.release()
            psum_sTb.release()
            psum_sTa.release()
            psum_proj.release()

    nc.finalize()
    return nc


def _get_nc(reps=1):
    key = f"nc{reps}"
    if key not in _CACHE:
        _CACHE[key] = _build_nc(reps)
    return _CACHE[key]


def pack_inputs(x, Wq, Wk, Wv):
    """Host-side prep: cast/pack full inputs into the concatenated per-core
    arrays the device program consumes (axis 0 = core shards)."""
    import ml_dtypes

    bf16 = ml_dtypes.bfloat16
    x = np.ascontiguousarray(np.asarray(x, dtype=np.float32)).astype(bf16)
    wall = np.concatenate(
        [np.asarray(Wq), np.asarray(Wk), np.asarray(Wv)], axis=1, dtype=np.float32
    )  # [D, 3H]; cols 0:128 = [Wq|Wk] (the stacked qk stationary), 128:192 = Wv
    wqkv = np.ascontiguousarray(
        wall.reshape(ND, P, 3 * H).transpose(1, 0, 2)
    ).astype(bf16)  # [P, ND, 3H]
    # mask[k, q] = 1.0 where q >= k (upper-tri incl diagonal, sT layout)
    mask = np.triu(np.ones((P, P), dtype=np.float32)).astype(bf16)
    return {
        "x": x.reshape(NCORES * T, D),                       # concat of x[b]
        "wqkv": np.concatenate([wqkv] * NCORES, axis=0),     # replicated
        "mask": np.concatenate([mask] * NCORES, axis=0),     # replicated
    }


def _get_runner(reps=1):
    """Build (once) the jitted shard_map executor for the bass program.

    Mirrors concourse.bass2jax.run_bass_via_pjrt's multi-core path, but the
    jitted callable is cached so repeat executions reuse the compiled
    executable instead of re-tracing/re-lowering per call. `reps` selects the
    N-repetition timing variant of the program (see _build_nc).
    """
    key = f"runner{reps}"
    if key in _CACHE:
        return _CACHE[key]

    import jax
    from jax.experimental.shard_map import shard_map
    from jax.sharding import Mesh, NamedSharding, PartitionSpec
    from concourse import bass2jax, mybir

    nc = _get_nc(reps)
    bass2jax.install_neuronx_cc_hook()

    partition_name = (
        nc.partition_id_tensor.name if nc.partition_id_tensor else None
    )
    in_names, out_names, out_avals, zero_shapes = [], [], [], []
    for alloc in nc.m.functions[0].allocations:
        if not isinstance(alloc, mybir.MemoryLocationSet):
            continue
        name = alloc.memorylocations[0].name
        if alloc.kind == "ExternalInput":
            if name != partition_name:
                in_names.append(name)
        elif alloc.kind == "ExternalOutput":
            out_names.append(name)
            shape = tuple(alloc.tensor_shape)
            dtype = mybir.dt.np(alloc.dtype)
            out_avals.append(jax.core.ShapedArray(shape, dtype))
            zero_shapes.append((shape, dtype))
    n_params = len(in_names)
    n_outs = len(out_avals)
    all_in_names = list(in_names) + list(out_names)
    if partition_name is not None:
        all_in_names.append(partition_name)

    def _body(*args):
        operands = list(args)
        if partition_name is not None:
            operands.append(bass2jax.partition_id_tensor())
        outs = bass2jax._bass_exec_p.bind(
            *operands,
            out_avals=tuple(out_avals),
            in_names=tuple(all_in_names),
            out_names=tuple(out_names),
            lowering_input_output_aliases=(),
            sim_require_finite=True,
            sim_require_nnan=True,
            nc=nc,
        )
        return tuple(outs)

    devices = jax.devices()[:NCORES]
    mesh = Mesh(np.asarray(devices), ("core",))
    spec = PartitionSpec("core")
    donate = tuple(range(n_params, n_params + n_outs))
    sharded = jax.jit(
        shard_map(
            _body,
            mesh=mesh,
            in_specs=(spec,) * (n_params + n_outs),
            out_specs=(spec,) * n_outs,
            check_rep=False,
        ),
        donate_argnums=donate,
        keep_unused=True,
    )
    runner = {
        "fn": sharded,
        "in_names": in_names,
        "out_names": out_names,
        "zero_shapes": zero_shapes,
        "sharding": NamedSharding(mesh, spec),
        "mesh": mesh,
    }
    _CACHE[key] = runner
    return runner


def new_zeros(runner=None):
    """Fresh (concatenated) zero output buffers — donated by each execute."""
    runner = runner or _get_runner()
    return [
        np.zeros((NCORES * s[0], *s[1:]), dt) for s, dt in runner["zero_shapes"]
    ]


def device_put_inputs(packed, runner=None):
    """Stage packed inputs on the cores with the execution sharding, so
    subsequent executes involve no host->device transfer for them."""
    import jax

    runner = runner or _get_runner()
    return {
        k: jax.device_put(v, runner["sharding"]) for k, v in packed.items()
    }


def execute(packed, zeros=None, runner=None):
    """One execution of the 8-core program. `packed` values may be numpy
    (uploaded per call) or device-resident arrays from device_put_inputs.
    Returns the raw concatenated bf16 output array (async jax.Array)."""
    runner = runner or _get_runner()
    if zeros is None:
        zeros = new_zeros(runner)
    args = [packed[name] for name in runner["in_names"]]
    outs = runner["fn"](*args, *zeros)
    return outs[0]


def kernel(x, Wq, Wk, Wv):
    packed = pack_inputs(x, Wq, Wk, Wv)
    out = np.asarray(execute(packed))  # [NCORES*T, H] bf16
    return out.reshape(NCORES, T, H).astype(np.float32)
